# revision 5
# baseline (speedup 1.0000x reference)
"""Balanced BCE loss with per-sample dynamic top-k negative mining on 8 TRN2 cores.

Math: for each sample the reference computes
    pos_count = sum(gt*mask), neg_raw = sum((1-gt)*mask)
    neg_count = min(neg_raw, 3*pos_count), k = int(neg_count)
    loss = BCE(pred, gt);  pos_loss = sum(loss*positive)
    neg_topk = sum of k largest loss*negative values
    per_sample = (pos_loss + neg_topk) / (pos_count + neg_count + eps); mean over N.

Every negative position has loss > 0 (p is bounded away from {0,1}), so the
neg_loss vector has exactly neg_raw nonzero entries.  Whenever
neg_raw <= 3*pos_count, k == neg_raw and the top-k sum equals the FULL sum of
negative losses.  The device kernel therefore computes 4 streaming reductions
per sample:
    A = sum(gt*mask)            B = sum(mask - gt*mask)
    C = sum(gt*mask*ln(p))      D = sum((mask - gt*mask)*ln(1-p))
and the host combines 16x4 scalars.  If a sample ever violates
neg_raw <= 3*pos_count, the host recomputes that sample exactly (numpy).

Device mapping: data-parallel over N, 2 samples/core.  Each [640,640] sample
is viewed as [128, 3200] and processed in 1600-wide chunks:
  - pred is DMAd as f32; gt/mask are 0.0/1.0 floats, so a stride-4 2-byte DMA
    of the high half of each f32 word loads them directly as bf16 (no cast
    pass; DMAd in 32-partition blocks to keep merged dims under the 16-bit
    ISA num_elem field).
  - ScalarE: ln(p) and ln(1-p) (= Ln(-1*p + 1) via activation scale/bias),
    bf16 out.
  - VectorE: per quantity one bf16 tensor_tensor product (2x mode) plus one
    bf16 tensor_scalar copy with accum_out (4x mode) that reduces it to a
    per-partition [128,1] column of a stats tile.
Output is one [128,16] stats tile -> host sums in float64.  bf16 is exact for
the 0/1 tensors and products with them, so the only rounding is on ln values
(~2^-9 relative per element, averaging out over ~100k elements).
"""

import sys

if "/opt/trn_rl_repo" not in sys.path:
    sys.path.insert(0, "/opt/trn_rl_repo")

import numpy as np

N, H, W = 16, 640, 640
NEG_RATIO = 3.0
EPS = 1e-8
N_CORES = 8
S = N // N_CORES          # samples per core
P = 128
FREE = H * W // P         # 3200
CHUNK = 1600              # free-dim chunk
NCHUNKS = FREE // CHUNK   # 2
NQ = 4                    # quantities A,B,C,D
PBLK = 32                 # partition block for strided bf16-extraction DMA

_STATE = {}


def _build():
    import concourse.bass as bass
    import concourse.tile as tile
    from concourse import bacc, mybir

    f32 = mybir.dt.float32
    bf16 = mybir.dt.bfloat16
    Alu = mybir.AluOpType
    Act = mybir.ActivationFunctionType

    nc = bacc.Bacc("TRN2", target_bir_lowering=False, debug=False,
                   num_devices=N_CORES)
    pred_d = nc.dram_tensor("pred", [S, H, W], f32, kind="ExternalInput").ap()
    # gt/mask arrive as f32 0.0/1.0; declared bf16 with doubled W so the
    # odd (high) halfwords can be DMA-gathered as bf16 values directly.
    gt_d = nc.dram_tensor("gt", [S, H, 2 * W], bf16,
                          kind="ExternalInput").ap()
    mask_d = nc.dram_tensor("mask", [S, H, 2 * W], bf16,
                            kind="ExternalInput").ap()
    stats_d = nc.dram_tensor("stats", [P, S * NQ * NCHUNKS], f32,
                             kind="ExternalOutput").ap()

    with tile.TileContext(nc) as tc:
        with tc.tile_pool(name="inp", bufs=3) as inp, \
             tc.tile_pool(name="mid", bufs=2) as mid, \
             tc.tile_pool(name="res", bufs=1) as res:
            stats = res.tile([P, S * NQ * NCHUNKS], f32)

            def col(s, q, c):
                i = (s * NQ + q) * NCHUNKS + c
                return stats[:, i:i + 1]

            for s in range(S):
                pred_v = pred_d[s].rearrange("(p a) w -> p (a w)", p=P)
                # [128, 3200, 2] bf16 view; [..., 1] = high halves
                gt_v = gt_d[s].rearrange("(p a) (w two) -> p (a w) two",
                                         p=P, two=2)
                mask_v = mask_d[s].rearrange("(p a) (w two) -> p (a w) two",
                                             p=P, two=2)
                for c in range(NCHUNKS):
                    sl = bass.ts(c, CHUNK)
                    tp = inp.tile([P, CHUNK], f32, tag="pred")
                    nc.sync.dma_start(tp[:], pred_v[:, sl])
                    g16 = inp.tile([P, CHUNK], bf16, tag="gt")
                    m16 = inp.tile([P, CHUNK], bf16, tag="mask")
                    for b in range(P // PBLK):
                        pb = slice(PBLK * b, PBLK * (b + 1))
                        nc.sync.dma_start(g16[pb, :], gt_v[pb, sl, 1])
                        nc.sync.dma_start(m16[pb, :], mask_v[pb, sl, 1])

                    lp = mid.tile([P, CHUNK], bf16, tag="lp")
                    nc.scalar.activation(lp[:], tp[:], Act.Ln)
                    l1p = mid.tile([P, CHUNK], bf16, tag="l1p")
                    nc.scalar.activation(l1p[:], tp[:], Act.Ln,
                                         bias=1.0, scale=-1.0)

                    gm = mid.tile([P, CHUNK], bf16, tag="gm")
                    nc.vector.tensor_tensor(gm[:], g16[:], m16[:], Alu.mult)
                    neg = mid.tile([P, CHUNK], bf16, tag="neg")
                    nc.vector.tensor_tensor(neg[:], m16[:], gm[:],
                                            Alu.subtract)
                    t1 = mid.tile([P, CHUNK], bf16, tag="t1")
                    nc.vector.tensor_tensor(t1[:], gm[:], lp[:], Alu.mult)
                    t2 = mid.tile([P, CHUNK], bf16, tag="t2")
                    nc.vector.tensor_tensor(t2[:], neg[:], l1p[:], Alu.mult)

                    for q, src in enumerate((gm, neg, t1, t2)):
                        scr = mid.tile([P, CHUNK], bf16, tag="scr",
                                       name=f"scr_{s}_{c}_{q}")
                        nc.vector.tensor_scalar(
                            scr[:], src[:], 1.0, 0.0, Alu.mult, Alu.add,
                            accum_out=col(s, q, c))

            nc.sync.dma_start(stats_d[:], stats[:])
    nc.compile()
    return nc


def _get_nc():
    if "nc" not in _STATE:
        _STATE["nc"] = _build()
    return _STATE["nc"]


def _host_topk_fallback(p, g, m):
    """Exact per-sample reference semantics in numpy (rare path)."""
    p = p.astype(np.float32)
    positive = g * m
    negative = (1.0 - g) * m
    pos_count = positive.sum(dtype=np.float64)
    neg_count = min(negative.sum(dtype=np.float64), pos_count * NEG_RATIO)
    log_p = np.maximum(np.log(p), -100.0)
    log_1mp = np.maximum(np.log1p(-p), -100.0)
    loss = -(g * log_p + (1.0 - g) * log_1mp)
    pos_loss_sum = (loss * positive).sum(dtype=np.float64)
    neg_loss = (loss * negative).ravel()
    k = int(neg_count)
    if k > 0:
        top = np.partition(neg_loss, len(neg_loss) - k)[len(neg_loss) - k:]
        neg_topk = top.sum(dtype=np.float64)
    else:
        neg_topk = 0.0
    return (pos_loss_sum + neg_topk) / (pos_count + neg_count + EPS)


def _combine(results, p, g, m):
    losses = []
    for c in range(N_CORES):
        o = results[c]["stats"].astype(np.float64)  # [128, S*NQ*NCHUNKS]
        sums = o.reshape(P, S, NQ, NCHUNKS).sum(axis=(0, 3))  # [S, NQ]
        for s in range(S):
            A, B, C, D = sums[s]
            pos_count = A
            neg_raw = B
            neg_count = min(neg_raw, pos_count * NEG_RATIO)
            k = int(neg_count)
            if k >= int(round(neg_raw)):
                # top-k covers every (strictly positive) negative loss
                losses.append((-C - D) / (pos_count + neg_count + EPS))
            else:
                i = c * S + s
                losses.append(_host_topk_fallback(p[i], g[i], m[i]))
    return np.float32(np.mean(losses))


def _in_maps(p, g, m):
    import ml_dtypes
    gv = g.view(np.uint16).view(ml_dtypes.bfloat16)  # [N, H, 2W]
    mv = m.view(np.uint16).view(ml_dtypes.bfloat16)
    return [
        {"pred": p[c * S:(c + 1) * S],
         "gt": gv[c * S:(c + 1) * S],
         "mask": mv[c * S:(c + 1) * S]}
        for c in range(N_CORES)
    ]


def kernel(pred, gt, mask):
    from concourse import bass_utils

    p = np.ascontiguousarray(pred[:, 0], dtype=np.float32)   # [N,H,W]
    g = np.ascontiguousarray(gt, dtype=np.float32)
    m = np.ascontiguousarray(mask, dtype=np.float32)

    nc = _get_nc()
    res = bass_utils.run_bass_kernel_spmd(nc, _in_maps(p, g, m),
                                          core_ids=list(range(N_CORES)))
    return _combine(res.results, p, g, m)


# revision 6
# speedup vs baseline: 36.8437x; 36.8437x over previous
"""Balanced BCE loss with per-sample dynamic top-k negative mining on 8 TRN2 cores.

Math: for each sample the reference computes
    pos_count = sum(gt*mask), neg_raw = sum((1-gt)*mask)
    neg_count = min(neg_raw, 3*pos_count), k = int(neg_count)
    loss = BCE(pred, gt);  pos_loss = sum(loss*positive)
    neg_topk = sum of k largest loss*negative values
    per_sample = (pos_loss + neg_topk) / (pos_count + neg_count + eps); mean over N.

Every negative position has loss > 0 (p is bounded away from {0,1}), so the
neg_loss vector has exactly neg_raw nonzero entries.  Whenever
neg_raw <= 3*pos_count, k == neg_raw and the top-k sum equals the FULL sum of
negative losses.  The device kernel therefore computes per sample
    A = sum(gt*mask)        M = sum(mask)          (B = M - A)
    C = sum(gt*mask*ln(p))  D = sum((mask-gt*mask)*ln(1-p))
and the host combines 16x4 scalars.  If a sample ever violates
neg_raw <= 3*pos_count, the host recomputes that sample exactly (numpy).

Device mapping: data-parallel over N, 2 samples/core.  Each [640,640] sample
is viewed as [128, 3200], processed in 1600-wide chunks:
  - ScalarE: ln(p), ln(1-p) (= Ln(-1*p+1) via activation scale/bias), bf16 out.
  - VectorE: m16 = bf16(mask) via tensor_scalar copy whose accum_out gives
    sum(mask) for free (f32 2-port 2x mode); gm = gt*mask as one f32
    tensor_tensor with bf16 output; neg/t1/t2 as bf16 tensor_tensor (2x);
    A = sum(gm) via bf16 tensor_scalar+accum (4x mode).
  - TensorE (idle otherwise): reduces t1 and t2 with a stationary ones[128,1]
    bf16 vector, accumulating [1,400] column sums in PSUM across the sample.
  - ScalarE copies the two PSUM rows to SBUF; DMA out.
Host sums the [128,8] count-stats and [S,2,400] loss partials in float64.
bf16 is exact for 0/1 tensors and products with them; only ln values are
rounded (~2^-9 relative, averaging out over ~100k summed elements).
"""

import sys

if "/opt/trn_rl_repo" not in sys.path:
    sys.path.insert(0, "/opt/trn_rl_repo")

import numpy as np

N, H, W = 16, 640, 640
NEG_RATIO = 3.0
EPS = 1e-8
N_CORES = 8
S = N // N_CORES          # samples per core
P = 128
FREE = H * W // P         # 3200
CHUNK = 1600              # free-dim chunk
NCHUNKS = FREE // CHUNK   # 2
MM = 400                  # matmul sub-chunk (PSUM bank: <=512 f32)
NMM = CHUNK // MM         # 4

_STATE = {}


def _build():
    import concourse.bass as bass
    import concourse.tile as tile
    from concourse import bacc, mybir

    f32 = mybir.dt.float32
    bf16 = mybir.dt.bfloat16
    Alu = mybir.AluOpType
    Act = mybir.ActivationFunctionType

    nc = bacc.Bacc("TRN2", target_bir_lowering=False, debug=False,
                   num_devices=N_CORES)
    pred_d = nc.dram_tensor("pred", [S, H, W], f32, kind="ExternalInput").ap()
    gt_d = nc.dram_tensor("gt", [S, H, W], f32, kind="ExternalInput").ap()
    mask_d = nc.dram_tensor("mask", [S, H, W], f32, kind="ExternalInput").ap()
    # per (sample, chunk): cols [A, M] per-partition partial counts
    stats_d = nc.dram_tensor("stats", [P, S * NCHUNKS * 2], f32,
                             kind="ExternalOutput").ap()
    cd_d = nc.dram_tensor("cd", [S, 2, MM], f32, kind="ExternalOutput").ap()

    with tile.TileContext(nc) as tc:
        with tc.tile_pool(name="cst", bufs=1) as cst, \
             tc.tile_pool(name="inp", bufs=3) as inp, \
             tc.tile_pool(name="mid", bufs=2) as mid, \
             tc.tile_pool(name="res", bufs=1) as res, \
             tc.tile_pool(name="ps", bufs=2, space="PSUM") as psp:
            ones = cst.tile([P, 1], bf16)
            nc.gpsimd.memset(ones[:], 1.0)
            stats = res.tile([P, S * NCHUNKS * 2], f32)

            def col(s, c, i):
                j = (s * NCHUNKS + c) * 2 + i
                return stats[:, j:j + 1]

            for s in range(S):
                pred_v = pred_d[s].rearrange("(p a) w -> p (a w)", p=P)
                gt_v = gt_d[s].rearrange("(p a) w -> p (a w)", p=P)
                mask_v = mask_d[s].rearrange("(p a) w -> p (a w)", p=P)
                accC = psp.tile([1, MM], f32, tag="accC", name=f"accC_{s}")
                accD = psp.tile([1, MM], f32, tag="accD", name=f"accD_{s}")
                for c in range(NCHUNKS):
                    sl = bass.ts(c, CHUNK)
                    tp = inp.tile([P, CHUNK], f32, tag="pred")
                    nc.sync.dma_start(tp[:], pred_v[:, sl])
                    tg = inp.tile([P, CHUNK], f32, tag="gt")
                    nc.sync.dma_start(tg[:], gt_v[:, sl])
                    tm = inp.tile([P, CHUNK], f32, tag="mask")
                    nc.sync.dma_start(tm[:], mask_v[:, sl])

                    lp = mid.tile([P, CHUNK], bf16, tag="lp")
                    nc.scalar.activation(lp[:], tp[:], Act.Ln)
                    l1p = mid.tile([P, CHUNK], bf16, tag="l1p")
                    nc.scalar.activation(l1p[:], tp[:], Act.Ln,
                                         bias=1.0, scale=-1.0)

                    # bf16 cast of mask; accum gives sum(mask) for free
                    m16 = mid.tile([P, CHUNK], bf16, tag="m16")
                    nc.vector.tensor_scalar(
                        m16[:], tm[:], 1.0, 0.0, Alu.mult, Alu.add,
                        accum_out=col(s, c, 1))
                    # gm = gt*mask, f32 inputs, bf16 out (1x mode)
                    gm = mid.tile([P, CHUNK], bf16, tag="gm")
                    nc.vector.tensor_tensor(gm[:], tg[:], tm[:], Alu.mult)
                    # A = sum(gm) (bf16 4x mode)
                    scrA = mid.tile([P, CHUNK], bf16, tag="scrA")
                    nc.vector.tensor_scalar(
                        scrA[:], gm[:], 1.0, 0.0, Alu.mult, Alu.add,
                        accum_out=col(s, c, 0))

                    neg = mid.tile([P, CHUNK], bf16, tag="neg")
                    nc.vector.tensor_tensor(neg[:], m16[:], gm[:],
                                            Alu.subtract)
                    t1 = mid.tile([P, CHUNK], bf16, tag="t1")
                    nc.vector.tensor_tensor(t1[:], gm[:], lp[:], Alu.mult)
                    t2 = mid.tile([P, CHUNK], bf16, tag="t2")
                    nc.vector.tensor_tensor(t2[:], neg[:], l1p[:], Alu.mult)

                    for m in range(NMM):
                        step = c * NMM + m
                        first = step == 0
                        last = step == NCHUNKS * NMM - 1
                        nc.tensor.matmul(accC[:], ones[:],
                                         t1[:, bass.ts(m, MM)],
                                         start=first, stop=last)
                        nc.tensor.matmul(accD[:], ones[:],
                                         t2[:, bass.ts(m, MM)],
                                         start=first, stop=last)

                for i, acc in enumerate((accC, accD)):
                    ot = mid.tile([1, MM], f32, tag="ot", name=f"ot{i}_{s}")
                    nc.scalar.copy(ot[:], acc[:])
                    nc.sync.dma_start(cd_d[s, i], ot[:])

            nc.sync.dma_start(stats_d[:], stats[:])
    nc.compile()
    return nc


def _get_nc():
    if "nc" not in _STATE:
        _STATE["nc"] = _build()
    return _STATE["nc"]


def _host_topk_fallback(p, g, m):
    """Exact per-sample reference semantics in numpy (rare path)."""
    p = p.astype(np.float32)
    positive = g * m
    negative = (1.0 - g) * m
    pos_count = positive.sum(dtype=np.float64)
    neg_count = min(negative.sum(dtype=np.float64), pos_count * NEG_RATIO)
    log_p = np.maximum(np.log(p), -100.0)
    log_1mp = np.maximum(np.log1p(-p), -100.0)
    loss = -(g * log_p + (1.0 - g) * log_1mp)
    pos_loss_sum = (loss * positive).sum(dtype=np.float64)
    neg_loss = (loss * negative).ravel()
    k = int(neg_count)
    if k > 0:
        top = np.partition(neg_loss, len(neg_loss) - k)[len(neg_loss) - k:]
        neg_topk = top.sum(dtype=np.float64)
    else:
        neg_topk = 0.0
    return (pos_loss_sum + neg_topk) / (pos_count + neg_count + EPS)


def _combine(results, p, g, m):
    losses = []
    for c in range(N_CORES):
        st = results[c]["stats"].astype(np.float64)  # [128, S*NCHUNKS*2]
        st = st.reshape(P, S, NCHUNKS, 2).sum(axis=(0, 2))  # [S, 2] = A, M
        cd = results[c]["cd"].astype(np.float64).sum(axis=2)  # [S, 2] = C, D
        for s in range(S):
            A, M = st[s]
            C, D = cd[s]
            pos_count = A
            neg_raw = M - A
            neg_count = min(neg_raw, pos_count * NEG_RATIO)
            k = int(neg_count)
            if k >= int(round(neg_raw)):
                # top-k covers every (strictly positive) negative loss
                losses.append((-C - D) / (pos_count + neg_count + EPS))
            else:
                i = c * S + s
                losses.append(_host_topk_fallback(p[i], g[i], m[i]))
    return np.float32(np.mean(losses))


def _in_maps(p, g, m):
    return [
        {"pred": p[c * S:(c + 1) * S],
         "gt": g[c * S:(c + 1) * S],
         "mask": m[c * S:(c + 1) * S]}
        for c in range(N_CORES)
    ]


def kernel(pred, gt, mask):
    from concourse import bass_utils

    p = np.ascontiguousarray(pred[:, 0], dtype=np.float32)   # [N,H,W]
    g = np.ascontiguousarray(gt, dtype=np.float32)
    m = np.ascontiguousarray(mask, dtype=np.float32)

    nc = _get_nc()
    res = bass_utils.run_bass_kernel_spmd(nc, _in_maps(p, g, m),
                                          core_ids=list(range(N_CORES)))
    return _combine(res.results, p, g, m)


# revision 7
# speedup vs baseline: 40.0290x; 1.0865x over previous
"""Balanced BCE loss with per-sample dynamic top-k negative mining on 8 TRN2 cores.

Math: for each sample the reference computes
    pos_count = sum(gt*mask), neg_raw = sum((1-gt)*mask)
    neg_count = min(neg_raw, 3*pos_count), k = int(neg_count)
    loss = BCE(pred, gt);  pos_loss = sum(loss*positive)
    neg_topk = sum of k largest loss*negative values
    per_sample = (pos_loss + neg_topk) / (pos_count + neg_count + eps); mean over N.

Every negative position has loss > 0 (p is bounded away from {0,1}), so the
neg_loss vector has exactly neg_raw nonzero entries.  Whenever
neg_raw <= 3*pos_count, k == neg_raw and the top-k sum equals the FULL sum of
negative losses.  The device kernel therefore computes per sample
    A = sum(gt*mask)        M = sum(mask)          (B = M - A)
    C = sum(gt*mask*ln(p))  D = sum((mask-gt*mask)*ln(1-p))
(A, C, D reduced on TensorE with a ones[128,1] stationary vector; M comes for
free from the accum_out of the ScalarE pass that casts mask to bf16.)
and the host combines 16x4 scalars.  If a sample ever violates
neg_raw <= 3*pos_count, the host recomputes that sample exactly (numpy).

Device mapping: data-parallel over N, 2 samples/core.  Each [640,640] sample
is viewed as [128, 3200], processed in 1600-wide chunks:
  - ScalarE: ln(p), ln(1-p) (= Ln(-1*p+1) via activation scale/bias), bf16 out.
  - VectorE: m16 = bf16(mask) via tensor_scalar copy whose accum_out gives
    sum(mask) for free (f32 2-port 2x mode); gm = gt*mask as one f32
    tensor_tensor with bf16 output; neg/t1/t2 as bf16 tensor_tensor (2x);
    A = sum(gm) via bf16 tensor_scalar+accum (4x mode).
  - TensorE (idle otherwise): reduces t1 and t2 with a stationary ones[128,1]
    bf16 vector, accumulating [1,400] column sums in PSUM across the sample.
  - ScalarE copies the two PSUM rows to SBUF; DMA out.
Host sums the [128,8] count-stats and [S,2,400] loss partials in float64.
bf16 is exact for 0/1 tensors and products with them; only ln values are
rounded (~2^-9 relative, averaging out over ~100k summed elements).
"""

import sys

if "/opt/trn_rl_repo" not in sys.path:
    sys.path.insert(0, "/opt/trn_rl_repo")

import numpy as np

N, H, W = 16, 640, 640
NEG_RATIO = 3.0
EPS = 1e-8
N_CORES = 8
S = N // N_CORES          # samples per core
P = 128
FREE = H * W // P         # 3200
CHUNK = 1600              # free-dim chunk
NCHUNKS = FREE // CHUNK   # 2
MM = 400                  # matmul sub-chunk (PSUM bank: <=512 f32)
NMM = CHUNK // MM         # 4

_STATE = {}


def _build():
    import concourse.bass as bass
    import concourse.tile as tile
    from concourse import bacc, mybir

    f32 = mybir.dt.float32
    bf16 = mybir.dt.bfloat16
    Alu = mybir.AluOpType
    Act = mybir.ActivationFunctionType

    nc = bacc.Bacc("TRN2", target_bir_lowering=False, debug=False,
                   num_devices=N_CORES)
    pred_d = nc.dram_tensor("pred", [S, H, W], f32, kind="ExternalInput").ap()
    gt_d = nc.dram_tensor("gt", [S, H, W], f32, kind="ExternalInput").ap()
    mask_d = nc.dram_tensor("mask", [S, H, W], f32, kind="ExternalInput").ap()
    # per (sample, chunk): one per-partition partial sum(mask) column
    stats_d = nc.dram_tensor("stats", [P, S * NCHUNKS], f32,
                             kind="ExternalOutput").ap()
    acd_d = nc.dram_tensor("acd", [S, 3, MM], f32, kind="ExternalOutput").ap()

    with tile.TileContext(nc) as tc:
        with tc.tile_pool(name="cst", bufs=1) as cst, \
             tc.tile_pool(name="inp", bufs=3) as inp, \
             tc.tile_pool(name="mid", bufs=2) as mid, \
             tc.tile_pool(name="res", bufs=1) as res, \
             tc.tile_pool(name="ps", bufs=2, space="PSUM") as psp:
            ones = cst.tile([P, 1], bf16)
            nc.gpsimd.memset(ones[:], 1.0)
            stats = res.tile([P, S * NCHUNKS], f32)

            def col(s, c):
                j = s * NCHUNKS + c
                return stats[:, j:j + 1]

            for s in range(S):
                pred_v = pred_d[s].rearrange("(p a) w -> p (a w)", p=P)
                gt_v = gt_d[s].rearrange("(p a) w -> p (a w)", p=P)
                mask_v = mask_d[s].rearrange("(p a) w -> p (a w)", p=P)
                accA = psp.tile([1, MM], f32, tag="accA", name=f"accA_{s}")
                accC = psp.tile([1, MM], f32, tag="accC", name=f"accC_{s}")
                accD = psp.tile([1, MM], f32, tag="accD", name=f"accD_{s}")
                for c in range(NCHUNKS):
                    sl = bass.ts(c, CHUNK)
                    tp = inp.tile([P, CHUNK], f32, tag="pred")
                    nc.sync.dma_start(tp[:], pred_v[:, sl])
                    tg = inp.tile([P, CHUNK], f32, tag="gt")
                    nc.sync.dma_start(tg[:], gt_v[:, sl])
                    tm = inp.tile([P, CHUNK], f32, tag="mask")
                    nc.sync.dma_start(tm[:], mask_v[:, sl])

                    lp = mid.tile([P, CHUNK], bf16, tag="lp")
                    nc.scalar.activation(lp[:], tp[:], Act.Ln)
                    l1p = mid.tile([P, CHUNK], bf16, tag="l1p")
                    nc.scalar.activation(l1p[:], tp[:], Act.Ln,
                                         bias=1.0, scale=-1.0)

                    # bf16 cast of mask on ScalarE; accum gives sum(mask)
                    m16 = mid.tile([P, CHUNK], bf16, tag="m16")
                    nc.scalar.activation(m16[:], tm[:], Act.Copy,
                                         accum_out=col(s, c))
                    # gm = gt*mask, f32 inputs, bf16 out (1x mode)
                    gm = mid.tile([P, CHUNK], bf16, tag="gm")
                    nc.vector.tensor_tensor(gm[:], tg[:], tm[:], Alu.mult)

                    neg = mid.tile([P, CHUNK], bf16, tag="neg")
                    nc.vector.tensor_tensor(neg[:], m16[:], gm[:],
                                            Alu.subtract)
                    t1 = mid.tile([P, CHUNK], bf16, tag="t1")
                    nc.vector.tensor_tensor(t1[:], gm[:], lp[:], Alu.mult)
                    t2 = mid.tile([P, CHUNK], bf16, tag="t2")
                    nc.vector.tensor_tensor(t2[:], neg[:], l1p[:], Alu.mult)

                    for m in range(NMM):
                        step = c * NMM + m
                        first = step == 0
                        last = step == NCHUNKS * NMM - 1
                        nc.tensor.matmul(accA[:], ones[:],
                                         gm[:, bass.ts(m, MM)],
                                         start=first, stop=last)
                        nc.tensor.matmul(accC[:], ones[:],
                                         t1[:, bass.ts(m, MM)],
                                         start=first, stop=last)
                        nc.tensor.matmul(accD[:], ones[:],
                                         t2[:, bass.ts(m, MM)],
                                         start=first, stop=last)

                for i, acc in enumerate((accA, accC, accD)):
                    ot = mid.tile([1, MM], f32, tag="ot", name=f"ot{i}_{s}")
                    nc.scalar.copy(ot[:], acc[:])
                    nc.sync.dma_start(acd_d[s, i], ot[:])

            nc.sync.dma_start(stats_d[:], stats[:])
    nc.compile()
    return nc


def _get_nc():
    if "nc" not in _STATE:
        _STATE["nc"] = _build()
    return _STATE["nc"]


def _host_topk_fallback(p, g, m):
    """Exact per-sample reference semantics in numpy (rare path)."""
    p = p.astype(np.float32)
    positive = g * m
    negative = (1.0 - g) * m
    pos_count = positive.sum(dtype=np.float64)
    neg_count = min(negative.sum(dtype=np.float64), pos_count * NEG_RATIO)
    log_p = np.maximum(np.log(p), -100.0)
    log_1mp = np.maximum(np.log1p(-p), -100.0)
    loss = -(g * log_p + (1.0 - g) * log_1mp)
    pos_loss_sum = (loss * positive).sum(dtype=np.float64)
    neg_loss = (loss * negative).ravel()
    k = int(neg_count)
    if k > 0:
        top = np.partition(neg_loss, len(neg_loss) - k)[len(neg_loss) - k:]
        neg_topk = top.sum(dtype=np.float64)
    else:
        neg_topk = 0.0
    return (pos_loss_sum + neg_topk) / (pos_count + neg_count + EPS)


def _combine(results, p, g, m):
    losses = []
    for c in range(N_CORES):
        st = results[c]["stats"].astype(np.float64)  # [128, S*NCHUNKS]
        st = st.reshape(P, S, NCHUNKS).sum(axis=(0, 2))      # [S] = M
        acd = results[c]["acd"].astype(np.float64).sum(axis=2)  # [S,3] A,C,D
        for s in range(S):
            M = st[s]
            A, C, D = acd[s]
            pos_count = A
            neg_raw = M - A
            neg_count = min(neg_raw, pos_count * NEG_RATIO)
            k = int(neg_count)
            if k >= int(round(neg_raw)):
                # top-k covers every (strictly positive) negative loss
                losses.append((-C - D) / (pos_count + neg_count + EPS))
            else:
                i = c * S + s
                losses.append(_host_topk_fallback(p[i], g[i], m[i]))
    return np.float32(np.mean(losses))


def _in_maps(p, g, m):
    return [
        {"pred": p[c * S:(c + 1) * S],
         "gt": g[c * S:(c + 1) * S],
         "mask": m[c * S:(c + 1) * S]}
        for c in range(N_CORES)
    ]


def kernel(pred, gt, mask):
    from concourse import bass_utils

    p = np.ascontiguousarray(pred[:, 0], dtype=np.float32)   # [N,H,W]
    g = np.ascontiguousarray(gt, dtype=np.float32)
    m = np.ascontiguousarray(mask, dtype=np.float32)

    nc = _get_nc()
    res = bass_utils.run_bass_kernel_spmd(nc, _in_maps(p, g, m),
                                          core_ids=list(range(N_CORES)))
    return _combine(res.results, p, g, m)


# revision 8
# speedup vs baseline: 42.8647x; 1.0708x over previous
"""Balanced BCE loss with per-sample dynamic top-k negative mining on 8 TRN2 cores.

Math: for each sample the reference computes
    pos_count = sum(gt*mask), neg_raw = sum((1-gt)*mask)
    neg_count = min(neg_raw, 3*pos_count), k = int(neg_count)
    loss = BCE(pred, gt);  pos_loss = sum(loss*positive)
    neg_topk = sum of k largest loss*negative values
    per_sample = (pos_loss + neg_topk) / (pos_count + neg_count + eps); mean over N.

Every negative position has loss > 0 (p is bounded away from {0,1}), so the
neg_loss vector has exactly neg_raw nonzero entries.  Whenever
neg_raw <= 3*pos_count, k == neg_raw and the top-k sum equals the FULL sum of
negative losses.  The device kernel therefore computes per sample
    A = sum(gt*mask)        M = sum(mask)          (B = M - A)
    C = sum(gt*mask*ln(p))  D = sum((mask-gt*mask)*ln(1-p))
(A, C, D reduced on TensorE with a ones[128,1] stationary vector; M comes for
free from the accum_out of the ScalarE pass that casts mask to bf16.)
and the host combines 16x4 scalars.  If a sample ever violates
neg_raw <= 3*pos_count, the host recomputes that sample exactly (numpy).

Device mapping: data-parallel over N, 2 samples/core.  Each [640,640] sample
is viewed as [128, 3200], processed in 1600-wide chunks:
  - ScalarE: ln(p), ln(1-p) (= Ln(-1*p+1) via activation scale/bias), bf16 out.
  - VectorE: m16 = bf16(mask) via tensor_scalar copy whose accum_out gives
    sum(mask) for free (f32 2-port 2x mode); gm = gt*mask as one f32
    tensor_tensor with bf16 output; neg/t1/t2 as bf16 tensor_tensor (2x);
    A = sum(gm) via bf16 tensor_scalar+accum (4x mode).
  - TensorE (idle otherwise): reduces t1 and t2 with a stationary ones[128,1]
    bf16 vector, accumulating [1,400] column sums in PSUM across the sample.
  - ScalarE copies the two PSUM rows to SBUF; DMA out.
Host sums the [128,8] count-stats and [S,2,400] loss partials in float64.
bf16 is exact for 0/1 tensors and products with them; only ln values are
rounded (~2^-9 relative, averaging out over ~100k summed elements).
"""

import sys

if "/opt/trn_rl_repo" not in sys.path:
    sys.path.insert(0, "/opt/trn_rl_repo")

import numpy as np

N, H, W = 16, 640, 640
NEG_RATIO = 3.0
EPS = 1e-8
N_CORES = 8
S = N // N_CORES          # samples per core
P = 128
FREE = H * W // P         # 3200
CHUNK = 800               # free-dim chunk
NCHUNKS = FREE // CHUNK   # 4
MM = 400                  # matmul sub-chunk (PSUM bank: <=512 f32)
NMM = CHUNK // MM         # 2

_STATE = {}


def _build():
    import concourse.bass as bass
    import concourse.tile as tile
    from concourse import bacc, mybir

    f32 = mybir.dt.float32
    bf16 = mybir.dt.bfloat16
    Alu = mybir.AluOpType
    Act = mybir.ActivationFunctionType

    nc = bacc.Bacc("TRN2", target_bir_lowering=False, debug=False,
                   num_devices=N_CORES)
    pred_d = nc.dram_tensor("pred", [S, H, W], f32, kind="ExternalInput").ap()
    gt_d = nc.dram_tensor("gt", [S, H, W], f32, kind="ExternalInput").ap()
    mask_d = nc.dram_tensor("mask", [S, H, W], f32, kind="ExternalInput").ap()
    # per (sample, chunk): one per-partition partial sum(mask) column
    stats_d = nc.dram_tensor("stats", [P, S * NCHUNKS], f32,
                             kind="ExternalOutput").ap()
    acd_d = nc.dram_tensor("acd", [S, 3, MM], f32, kind="ExternalOutput").ap()

    with tile.TileContext(nc) as tc:
        with tc.tile_pool(name="cst", bufs=1) as cst, \
             tc.tile_pool(name="inp", bufs=4) as inp, \
             tc.tile_pool(name="mid", bufs=3) as mid, \
             tc.tile_pool(name="res", bufs=1) as res, \
             tc.tile_pool(name="ps", bufs=2, space="PSUM") as psp:
            ones = cst.tile([P, 1], bf16)
            nc.gpsimd.memset(ones[:], 1.0)
            stats = res.tile([P, S * NCHUNKS], f32)

            def col(s, c):
                j = s * NCHUNKS + c
                return stats[:, j:j + 1]

            for s in range(S):
                pred_v = pred_d[s].rearrange("(p a) w -> p (a w)", p=P)
                gt_v = gt_d[s].rearrange("(p a) w -> p (a w)", p=P)
                mask_v = mask_d[s].rearrange("(p a) w -> p (a w)", p=P)
                accA = psp.tile([1, MM], f32, tag="accA", name=f"accA_{s}")
                accC = psp.tile([1, MM], f32, tag="accC", name=f"accC_{s}")
                accD = psp.tile([1, MM], f32, tag="accD", name=f"accD_{s}")
                for c in range(NCHUNKS):
                    sl = bass.ts(c, CHUNK)
                    tp = inp.tile([P, CHUNK], f32, tag="pred")
                    nc.sync.dma_start(tp[:], pred_v[:, sl])
                    tg = inp.tile([P, CHUNK], f32, tag="gt")
                    nc.sync.dma_start(tg[:], gt_v[:, sl])
                    tm = inp.tile([P, CHUNK], f32, tag="mask")
                    nc.sync.dma_start(tm[:], mask_v[:, sl])

                    lp = mid.tile([P, CHUNK], bf16, tag="lp")
                    nc.scalar.activation(lp[:], tp[:], Act.Ln)
                    l1p = mid.tile([P, CHUNK], bf16, tag="l1p")
                    nc.scalar.activation(l1p[:], tp[:], Act.Ln,
                                         bias=1.0, scale=-1.0)

                    # bf16 cast of mask on ScalarE; accum gives sum(mask)
                    m16 = mid.tile([P, CHUNK], bf16, tag="m16")
                    nc.scalar.activation(m16[:], tm[:], Act.Copy,
                                         accum_out=col(s, c))
                    # gm = gt*mask, f32 inputs, bf16 out (1x mode)
                    gm = mid.tile([P, CHUNK], bf16, tag="gm")
                    nc.vector.tensor_tensor(gm[:], tg[:], tm[:], Alu.mult)

                    neg = mid.tile([P, CHUNK], bf16, tag="neg")
                    nc.vector.tensor_tensor(neg[:], m16[:], gm[:],
                                            Alu.subtract)
                    t1 = mid.tile([P, CHUNK], bf16, tag="t1")
                    nc.vector.tensor_tensor(t1[:], gm[:], lp[:], Alu.mult)
                    t2 = mid.tile([P, CHUNK], bf16, tag="t2")
                    nc.vector.tensor_tensor(t2[:], neg[:], l1p[:], Alu.mult)

                    for m in range(NMM):
                        step = c * NMM + m
                        first = step == 0
                        last = step == NCHUNKS * NMM - 1
                        nc.tensor.matmul(accA[:], ones[:],
                                         gm[:, bass.ts(m, MM)],
                                         start=first, stop=last)
                        nc.tensor.matmul(accC[:], ones[:],
                                         t1[:, bass.ts(m, MM)],
                                         start=first, stop=last)
                        nc.tensor.matmul(accD[:], ones[:],
                                         t2[:, bass.ts(m, MM)],
                                         start=first, stop=last)

                for i, acc in enumerate((accA, accC, accD)):
                    ot = mid.tile([1, MM], f32, tag="ot", name=f"ot{i}_{s}")
                    nc.vector.tensor_copy(ot[:], acc[:])
                    nc.sync.dma_start(acd_d[s, i], ot[:])

            nc.sync.dma_start(stats_d[:], stats[:])
    nc.compile()
    return nc


def _get_nc():
    if "nc" not in _STATE:
        _STATE["nc"] = _build()
    return _STATE["nc"]


def _host_topk_fallback(p, g, m):
    """Exact per-sample reference semantics in numpy (rare path)."""
    p = p.astype(np.float32)
    positive = g * m
    negative = (1.0 - g) * m
    pos_count = positive.sum(dtype=np.float64)
    neg_count = min(negative.sum(dtype=np.float64), pos_count * NEG_RATIO)
    log_p = np.maximum(np.log(p), -100.0)
    log_1mp = np.maximum(np.log1p(-p), -100.0)
    loss = -(g * log_p + (1.0 - g) * log_1mp)
    pos_loss_sum = (loss * positive).sum(dtype=np.float64)
    neg_loss = (loss * negative).ravel()
    k = int(neg_count)
    if k > 0:
        top = np.partition(neg_loss, len(neg_loss) - k)[len(neg_loss) - k:]
        neg_topk = top.sum(dtype=np.float64)
    else:
        neg_topk = 0.0
    return (pos_loss_sum + neg_topk) / (pos_count + neg_count + EPS)


def _combine(results, p, g, m):
    losses = []
    for c in range(N_CORES):
        st = results[c]["stats"].astype(np.float64)  # [128, S*NCHUNKS]
        st = st.reshape(P, S, NCHUNKS).sum(axis=(0, 2))      # [S] = M
        acd = results[c]["acd"].astype(np.float64).sum(axis=2)  # [S,3] A,C,D
        for s in range(S):
            M = st[s]
            A, C, D = acd[s]
            pos_count = A
            neg_raw = M - A
            neg_count = min(neg_raw, pos_count * NEG_RATIO)
            k = int(neg_count)
            if k >= int(round(neg_raw)):
                # top-k covers every (strictly positive) negative loss
                losses.append((-C - D) / (pos_count + neg_count + EPS))
            else:
                i = c * S + s
                losses.append(_host_topk_fallback(p[i], g[i], m[i]))
    return np.float32(np.mean(losses))


def _in_maps(p, g, m):
    return [
        {"pred": p[c * S:(c + 1) * S],
         "gt": g[c * S:(c + 1) * S],
         "mask": m[c * S:(c + 1) * S]}
        for c in range(N_CORES)
    ]


def kernel(pred, gt, mask):
    from concourse import bass_utils

    p = np.ascontiguousarray(pred[:, 0], dtype=np.float32)   # [N,H,W]
    g = np.ascontiguousarray(gt, dtype=np.float32)
    m = np.ascontiguousarray(mask, dtype=np.float32)

    nc = _get_nc()
    res = bass_utils.run_bass_kernel_spmd(nc, _in_maps(p, g, m),
                                          core_ids=list(range(N_CORES)))
    return _combine(res.results, p, g, m)
